# revision 26
# baseline (speedup 1.0000x reference)
"""Trainium2 Bass kernel for MACE-style message-passing convolution.

Reference computation (per edge e with sender s, receiver r):
    msg0 = node_feats[s]                          # [64] scalars
    u    = vectors[e] / |vectors[e]|
    Y1   = sqrt(3) u ;  Y2 = 5 quadratic harmonics of u
    mix  = MLP(radial[e])                         # [192] = m0|m1|m2
    msg  = [msg0*m0, (msg0 (x) Y1)*m1, (msg0 (x) Y2)*m2]   # [576]
    out[r] += msg / sqrt(16)

Strategy (8 NeuronCores, SPMD):
  * Host: sort edges by receiver, shard NODES across cores (core c owns
    nodes [2048c, 2048(c+1))) so each core gets a contiguous slice of
    sorted edges -> no collective needed.
  * Host bakes data layouts only (gather of node_feats by sender, edge
    window layout, one-hot scatter matrices, weight folding). All FLOPs
    of the reference run on-device.
  * Device per core, per 1024-edge chunk (8 windows of 128 edges):
    - feature-major MLP (bf16 matmuls, silu on ACT)
    - edge-major mix via stationary-activations matmul (bf16)
    - tensor product on DVE in 2x packed mode: msg columns are K-MAJOR
      per window (k*64+c; host un-permutes at the end) so the Y factor
      is read from a small 8-slot replicated tile (built once per 4
      chunks on ACT; GPSIMD is avoided because it shares an SBUF port
      with the DVE and demotes concurrent 2-port DVE ops to 1x) and
      every AP keeps long unit-stride innermost runs -> DVE 2x
      everywhere.
    - segment-sum via one-hot bf16 matmuls with CONTIGUOUS moving
      operands (strided rhs runs 4x slow on PE), accumulating in PSUM
      per 128-node block.
    - mix-PSUM evacuation split between ACT (g0) and a fused DVE
      multiply straight from PSUM (g1) to balance engines.
"""

import os
import sys
from contextlib import ExitStack

import numpy as np

sys.path.insert(0, "/opt/trn_rl_repo")

import ml_dtypes  # noqa: E402

import concourse.bass as bass  # noqa: E402
import concourse.bacc as bacc  # noqa: E402
import concourse.tile as tile  # noqa: E402
from concourse import mybir  # noqa: E402

N_CORES = 8
N_NODES = 16384
N_EDGES = 262144
MUL = 64
N_BASIS = 8
HIDDEN = 64
NUM_IRREPS = 3 * MUL  # 192
MSG_W = 9 * MUL  # 576
NODES_PER_CORE = N_NODES // N_CORES  # 2048
BLOCKS = NODES_PER_CORE // 128  # 16 node-blocks of 128
WIN = 128  # edges per window (matmul K)
CHUNK_E = 1024  # edges per MLP chunk (2 stacked groups of 512)

F32 = mybir.dt.float32
F16 = mybir.dt.float16
BF16 = mybir.dt.bfloat16
F32R = mybir.dt.float32r
AF = mybir.ActivationFunctionType
OP = mybir.AluOpType

BF = ml_dtypes.bfloat16


def _silu_norm():
    x = np.linspace(-12.0, 12.0, 24001)
    p = np.exp(-0.5 * x * x) / np.sqrt(2.0 * np.pi)
    s = x / (1.0 + np.exp(-x))
    trapz = getattr(np, "trapz", None) or np.trapezoid
    return float(1.0 / np.sqrt(trapz(s * s * p, x)))


def _prep(vectors, node_feats, radial_embedding, W0, W1, W2, W3,
          senders, receivers):
    """Host-side data marshaling: sort/shard/pad/bake layouts."""
    E = vectors.shape[0]
    snd = np.asarray(senders).astype(np.int64)
    rcv = np.asarray(receivers).astype(np.int64)
    vectors = np.asarray(vectors, dtype=np.float32)
    node_feats = np.asarray(node_feats, dtype=np.float32)
    radial = np.asarray(radial_embedding, dtype=np.float32)

    perm = np.argsort(rcv, kind="stable")
    rcv_s = rcv[perm]
    snd_s = snd[perm]
    v_s = vectors[perm]
    rad_s = radial[perm]

    bounds = np.searchsorted(rcv_s, np.arange(N_CORES + 1) * NODES_PER_CORE)
    e_counts = np.diff(bounds)
    E_pad = int(np.ceil(e_counts.max() / CHUNK_E) * CHUNK_E)
    W = E_pad // WIN  # windows per core
    CH = E_pad // CHUNK_E

    sn = _silu_norm()
    W0e = (np.asarray(W0, np.float32) / np.sqrt(N_BASIS))
    W1e = (np.asarray(W1, np.float32) * sn / np.sqrt(HIDDEN))
    W2e = (np.asarray(W2, np.float32) * sn / np.sqrt(HIDDEN))
    W3e = (np.asarray(W3, np.float32) * sn / np.sqrt(HIDDEN) / 4.0).copy()
    W3e[:, MUL:2 * MUL] *= np.sqrt(3.0)  # fold Y1 = sqrt(3) u

    def blockdiag(w):
        k, m = w.shape
        out = np.zeros((2 * k, 2 * m), np.float32)
        out[:k, :m] = w
        out[k:, m:] = w
        return out

    w01 = blockdiag(W0e).astype(BF)
    w1b = blockdiag(W1e).astype(BF)
    w2b = blockdiag(W2e).astype(BF)
    w3e = W3e.astype(BF)

    # Per-core block->window ranges, unified across cores (SPMD: one program)
    core = {}
    blo_all = np.full((N_CORES, BLOCKS), 10**9, np.int64)
    bhi_all = np.full((N_CORES, BLOCKS), -1, np.int64)
    for c in range(N_CORES):
        lo, hi = bounds[c], bounds[c + 1]
        ec = hi - lo
        rl = rcv_s[lo:hi] - c * NODES_PER_CORE  # local node ids [0, 2048)
        rl_pad = np.full(E_pad, -1, np.int64)
        rl_pad[:ec] = rl
        # block edge ranges within this core's (padded) edge list
        bb = np.searchsorted(rl, np.arange(BLOCKS + 1) * 128)
        for b in range(BLOCKS):
            if bb[b + 1] > bb[b]:
                blo_all[c, b] = bb[b] // WIN
                bhi_all[c, b] = (bb[b + 1] - 1) // WIN
        core[c] = dict(lo=lo, hi=hi, ec=ec, rl_pad=rl_pad)
    B_LO = blo_all.min(axis=0)
    B_HI = bhi_all.max(axis=0)
    for b in range(BLOCKS):
        if B_HI[b] < B_LO[b]:
            B_LO[b], B_HI[b] = 0, -1  # empty everywhere -> memset path
    # pair list in window-major emission order
    pairs = []  # (w, b, start, stop)
    for w in range(W):
        for b in range(BLOCKS):
            if B_LO[b] <= w <= B_HI[b]:
                pairs.append((w, b, w == B_LO[b], w == B_HI[b]))
    n_pairs = len(pairs)

    in_maps = []
    for c in range(N_CORES):
        cc = core[c]
        lo, ec = cc["lo"], cc["ec"]
        # padded per-core edge arrays
        v_pad = np.zeros((E_pad, 3), np.float32)
        v_pad[:, 0] = 1.0
        v_pad[:ec] = v_s[lo:lo + ec]
        rad_pad = np.zeros((E_pad, N_BASIS), np.float32)
        rad_pad[:ec] = rad_s[lo:lo + ec]
        snd_pad = np.zeros(E_pad, np.int64)
        snd_pad[:ec] = snd_s[lo:lo + ec]

        # msg0 in per-chunk (w8, c64) layout: [128, CH*512]
        msg0 = node_feats[snd_pad]  # [E_pad, 64] host gather (layout only)
        msg0 = (msg0.reshape(CH, 8, 128, MUL).transpose(2, 0, 1, 3)
                .reshape(128, CH * 8 * MUL).astype(BF))

        # vectors in (k3, w) layout: [128, 3*W]
        vint = (v_pad.reshape(W, WIN, 3).transpose(1, 2, 0)
                .reshape(128, 3 * W))

        r4 = rad_pad.reshape(CH, 2, 512, N_BASIS)
        rad16 = np.ascontiguousarray(
            r4.transpose(1, 3, 0, 2).reshape(16, CH * 512)).astype(BF)

        # one-hot scatter matrices per (w, b) pair, bf16 (exact 0/1)
        rlp = cc["rl_pad"]
        ohs = np.zeros((n_pairs, WIN, 128), BF)
        ar = np.arange(128)
        for i, (w, b, _, _) in enumerate(pairs):
            rloc = rlp[w * WIN:(w + 1) * WIN] - 128 * b
            ohs[i] = (rloc[:, None] == ar[None, :]).astype(BF)
        ohs = ohs.transpose(1, 0, 2).reshape(WIN, n_pairs * 128)

        in_maps.append({
            "msg0": np.ascontiguousarray(msg0),
            "vint": np.ascontiguousarray(vint),
            "rad16": np.ascontiguousarray(rad16),
            "ohs": np.ascontiguousarray(ohs),
            "w01": w01, "w1b": w1b, "w2b": w2b, "w3e": w3e,
        })

    meta = dict(W=W, CH=CH, pairs=pairs, n_pairs=n_pairs,
                B_LO=B_LO, B_HI=B_HI)
    return in_maps, meta


def _build(meta, sim_safe=False):
    """Build the SPMD Bass/Tile program (identical across cores).

    sim_safe: CoreSim doesn't implement the Silu ACT function; emit
    Sigmoid + elementwise multiply instead (identical math) for sim runs.
    """
    W = meta["W"]
    CH = meta["CH"]
    pairs = meta["pairs"]
    n_pairs = meta["n_pairs"]

    nc = bacc.Bacc("TRN2", target_bir_lowering=False, debug=False)
    msg0_d = nc.declare_dram_parameter("msg0", [128, CH * 512], BF16,
                                       isOutput=False)
    vint_d = nc.declare_dram_parameter("vint", [128, 3 * W], F32,
                                       isOutput=False)
    rad_d = nc.declare_dram_parameter("rad16", [16, CH * 512], BF16,
                                      isOutput=False)
    ohs_d = nc.declare_dram_parameter("ohs", [128, n_pairs * 128], BF16,
                                      isOutput=False)
    w01_d = nc.declare_dram_parameter("w01", [16, 128], BF16, isOutput=False)
    w1b_d = nc.declare_dram_parameter("w1b", [128, 128], BF16, isOutput=False)
    w2b_d = nc.declare_dram_parameter("w2b", [128, 128], BF16, isOutput=False)
    w3e_d = nc.declare_dram_parameter("w3e", [64, NUM_IRREPS], BF16,
                                      isOutput=False)
    out_d = nc.declare_dram_parameter("out", [NODES_PER_CORE, MSG_W], F32,
                                      isOutput=True)

    C15 = float(np.sqrt(15.0))
    C5H = float(np.sqrt(5.0) / 2.0)

    def silu(out_ap, in_ap):
        if sim_safe:
            nc.scalar.activation(out_ap, in_ap, AF.Sigmoid)
            nc.vector.tensor_tensor(out_ap, out_ap, in_ap, OP.mult)
        else:
            nc.scalar.activation(out_ap, in_ap, AF.Silu)

    with tile.TileContext(nc) as tc, ExitStack() as ctx:
        const = ctx.enter_context(tc.tile_pool(name="const", bufs=1))
        sphp = ctx.enter_context(tc.tile_pool(name="sph", bufs=1))
        radp = ctx.enter_context(tc.tile_pool(name="rad", bufs=2))
        hp = ctx.enter_context(tc.tile_pool(name="hp", bufs=2, space="PSUM"))
        hact = ctx.enter_context(tc.tile_pool(name="hact", bufs=2))
        h3p = ctx.enter_context(tc.tile_pool(name="h3", bufs=2))
        mixp = ctx.enter_context(tc.tile_pool(name="mixp", bufs=1,
                                              space="PSUM"))
        mxsp = ctx.enter_context(tc.tile_pool(name="mxs", bufs=2))
        abp = ctx.enter_context(tc.tile_pool(name="ab", bufs=4))
        m0p = ctx.enter_context(tc.tile_pool(name="m0", bufs=3))
        yrp = ctx.enter_context(tc.tile_pool(name="yr", bufs=2))
        msgp = ctx.enter_context(tc.tile_pool(name="msg", bufs=4))
        ohp = ctx.enter_context(tc.tile_pool(name="oh", bufs=2))
        aggp = ctx.enter_context(tc.tile_pool(name="agg", bufs=2,
                                              space="PSUM"))
        outp = ctx.enter_context(tc.tile_pool(name="outs", bufs=2))

        # --- head DMAs: vint first (longest dependency chain: sph) ---
        vt = sphp.tile([128, 3 * W], F32)
        nc.sync.dma_start(vt[:], vint_d[:])
        w01t = const.tile([16, 128], BF16)
        nc.sync.dma_start(w01t[:], w01_d[:])
        w1bt = const.tile([128, 128], BF16)
        nc.sync.dma_start(w1bt[:], w1b_d[:])
        w2bt = const.tile([128, 128], BF16)
        nc.sync.dma_start(w2bt[:], w2b_d[:])
        # two copies of W3 (partitions 0:64 and 64:128) so the mix matmul's
        # lhsT (h3 slice) and rhs share a base partition
        w3et = const.tile([128, NUM_IRREPS], BF16)
        nc.sync.dma_start(w3et[0:64, :], w3e_d[:])
        nc.sync.dma_start(w3et[64:128, :], w3e_d[:])

        # --- PE warm-up: the HAM clock gate keeps the PE at 1.2 GHz until
        # it sees ~3.4us of sustained activity. The PE is otherwise idle
        # during the head DMAs + sph phase, so the real matmul stream would
        # start cold. Burn ~4us of dummy matmuls on the already-loaded
        # weights (recycling the MLP PSUM pool: no extra banks).
        for wi in range(80):
            wps = hp.tile([128, 512], F32, tag="hps", name=f"warm{wi}")
            nc.tensor.matmul(wps[:, 0:128], w1bt[:], w2bt[:])

        # --- spherical harmonics, fp32, once over all windows ---
        # (k, w) layout: component-major, window innermost
        vsq = sphp.tile([128, 3 * W], F32)
        nc.vector.tensor_tensor(vsq[:], vt[:], vt[:], OP.mult)
        s2 = sphp.tile([128, W], F32)
        nc.vector.tensor_tensor(s2[:], vsq[:, 0:W], vsq[:, W:2 * W], OP.add)
        nc.vector.tensor_tensor(s2[:], s2[:], vsq[:, 2 * W:3 * W], OP.add)
        rs = sphp.tile([128, W], F32)
        nc.vector.reciprocal(rs[:], s2[:])
        rinv = sphp.tile([128, W], F32)  # 1/|v|
        nc.scalar.activation(rinv[:], rs[:], AF.Sqrt)
        u3 = sphp.tile([128, 3 * W], F32)
        nc.vector.tensor_tensor(
            u3[:].rearrange("p (k w) -> p k w", k=3),
            vt[:].rearrange("p (k w) -> p k w", k=3),
            rinv[:].unsqueeze(1).broadcast_to([128, 3, W]),
            OP.mult)
        ux = u3[:, 0:W]
        uy = u3[:, W:2 * W]
        uz = u3[:, 2 * W:3 * W]
        y5 = sphp.tile([128, 5 * W], F32)
        nc.vector.scalar_tensor_tensor(y5[:, 0:W], ux, C15, uy,
                                       OP.mult, OP.mult)
        nc.vector.scalar_tensor_tensor(y5[:, W:2 * W], uy, C15, uz,
                                       OP.mult, OP.mult)
        nc.vector.scalar_tensor_tensor(y5[:, 2 * W:3 * W], uz, 3.0 * C5H, uz,
                                       OP.mult, OP.mult)
        nc.vector.tensor_scalar_add(y5[:, 2 * W:3 * W], y5[:, 2 * W:3 * W],
                                    -C5H)
        nc.vector.scalar_tensor_tensor(y5[:, 3 * W:4 * W], ux, C15, uz,
                                       OP.mult, OP.mult)
        tpq = sphp.tile([128, 2 * W], F32)
        nc.vector.tensor_tensor(tpq[:, :W], ux, uy, OP.add)
        nc.vector.tensor_tensor(tpq[:, W:], ux, uy, OP.subtract)
        nc.vector.scalar_tensor_tensor(y5[:, 4 * W:5 * W], tpq[:, :W],
                                       C15 / 2.0, tpq[:, W:],
                                       OP.mult, OP.mult)

        # bf16 Y factors, all 8 components in one (k8, W) tile
        yall = sphp.tile([128, 8 * W], BF16)
        nc.vector.tensor_copy(yall[:, 0:3 * W], u3[:])
        nc.vector.tensor_copy(yall[:, 3 * W:8 * W], y5[:])
        yv = yall[:].rearrange("p (k w) -> p k w", k=8)

        # --- software-pipelined chunk loop ---
        LOOKAHEAD = 3
        pair_i = 0
        agg_a = {}
        agg_b = {}
        ab_tiles = {}
        msg_tiles = {}
        yrep = [None, None]

        def chunk_body(j):
            # MLP chunk j: 1024 edges as 2 stacked groups (even/odd windows
            # so consecutive mix matmuls alternate PE row-halves)
            radt = radp.tile([16, 512], BF16, tag="radt")
            nc.sync.dma_start(radt[:], rad_d[:, j * 512:(j + 1) * 512])
            h1ps = hp.tile([128, 512], F32, tag="hps")
            nc.tensor.matmul(h1ps[:], w01t[:], radt[:])
            h1 = hact.tile([128, 512], BF16, tag="h12")
            silu(h1[:], h1ps[:])
            h2ps = hp.tile([128, 512], F32, tag="hps")
            nc.tensor.matmul(h2ps[:], w1bt[:], h1[:])
            h2 = hact.tile([128, 512], BF16, tag="h12")
            silu(h2[:], h2ps[:])
            h3ps = hp.tile([128, 512], F32, tag="hps")
            nc.tensor.matmul(h3ps[:], w2bt[:], h2[:])
            h3 = h3p.tile([128, 512], BF16)
            silu(h3[:], h3ps[:])
            hoff = 0

            # msg0 chunk in (w8, c64) layout
            m0t = m0p.tile([128, 512], BF16)
            nc.sync.dma_start(
                m0t[:], msg0_d[:, j * 512:(j + 1) * 512])
            m0v = m0t[:].rearrange("p (w c) -> p w c", w=8)

            # 8-slot replicated Y factor tiles, built once per 4 chunks on
            # ACT (any-AP 1x; the big FD amortizes the per-op constant).
            # GPSIMD is NOT used: it shares an SBUF port with the DVE and
            # demotes concurrent 2-port (2x) DVE ops to 1x.
            if j % 4 == 0:
                wlo, whi = j * 8, min((j + 4) * 8, W)
                nw = whi - wlo
                yu8 = yrp.tile([128, 32 * 3 * 8], BF16, tag="yu8",
                               name=f"yu8_{j}")
                nc.scalar.activation(
                    yu8[:, :nw * 24].rearrange(
                        "p (w k s) -> p w k s", k=3, s=8),
                    yv[:, 0:3, wlo:whi].transpose([0, 2, 1])
                    .unsqueeze(3).broadcast_to([128, nw, 3, 8]),
                    AF.Copy)
                yy8 = yrp.tile([128, 32 * 5 * 8], BF16, tag="yy8",
                               name=f"yy8_{j}")
                nc.scalar.activation(
                    yy8[:, :nw * 40].rearrange(
                        "p (w k s) -> p w k s", k=5, s=8),
                    yv[:, 3:8, wlo:whi].transpose([0, 2, 1])
                    .unsqueeze(3).broadcast_to([128, nw, 5, 8]),
                    AF.Copy)
                yrep[0] = yu8
                yrep[1] = yy8

            # ab tile: (w8, l3, c64) layout
            ab = abp.tile([128, 8 * 3 * MUL], BF16, tag="ab",
                          name=f"ab_{j}", bufs=LOOKAHEAD + 2)
            ab_tiles[j] = ab
            abv = ab[:].rearrange("p (w l c) -> p w l c", w=8, l=3)

            for g in range(2):  # half-chunks of 4 windows
                # mix: edge-major via stationary-h3 trick; 256-col PSUM
                # slots so each matmul output stays inside one bank
                mixt = mixp.tile([128, 4 * 256], F32, tag="mixt")
                for t4 in range(4):
                    t = g * 4 + t4
                    half, coff = ((0, t * 128) if t < 4
                                  else (64, (t - 4) * 128))
                    nc.tensor.matmul(
                        mixt[:, t4 * 256:t4 * 256 + NUM_IRREPS],
                        h3[half:half + 64, coff:coff + 128],
                        w3et[half:half + 64, :])
                # PSUM mix view (w4, l, c)
                mixv = (mixt[:].rearrange("p (w x) -> p w x", x=256)
                        [:, :, 0:NUM_IRREPS]
                        .rearrange("p w (l c) -> p w l c", l=3))
                abw = abv[:, g * 4:(g + 1) * 4]
                m0w = (m0v[:, g * 4:(g + 1) * 4]
                       .unsqueeze(2).broadcast_to([128, 4, 3, MUL]))
                if g == 0:
                    # ACT evacuates PSUM -> bf16 SBUF (frees the single mix
                    # PSUM buffer fast); DVE multiplies at 2x
                    mixs = mxsp.tile([128, 4 * 3 * MUL], BF16, tag="mixs")
                    msv = mixs[:].rearrange("p (w l c) -> p w l c",
                                            w=4, l=3)
                    nc.scalar.activation(msv, mixv, AF.Copy)
                    nc.vector.tensor_tensor(abw, msv, m0w, OP.mult)
                else:
                    # fused: DVE reads mix from PSUM (1x) and multiplies
                    nc.vector.tensor_tensor(abw, mixv, m0w, OP.mult)

            # msg tile [128, 8*512]: (w8, col512) with K-MAJOR columns
            # col = k*64+c for l1 (0:192), 192 + k*64+c for l2 (192:512)
            msgt = msgp.tile([128, 8 * 512], BF16, bufs=LOOKAHEAD + 2)
            msg_tiles[j] = msgt
            l1v = (msgt[:].rearrange("p (w col) -> p w col", w=8)
                   [:, :, 0:192].rearrange("p w (k c) -> p w k c", k=3))
            l2v = (msgt[:].rearrange("p (w col) -> p w col", w=8)
                   [:, :, 192:512].rearrange("p w (k c) -> p w k c", k=5))
            ou = (j % 4) * 8 * 24
            oy = (j % 4) * 8 * 40
            yu_v = (yrep[0][:, ou:ou + 192]
                    .rearrange("p (wk s) -> p wk s", s=8)
                    .unsqueeze(2).broadcast_to([128, 24, 8, 8]))
            yy_v = (yrep[1][:, oy:oy + 320]
                    .rearrange("p (wk s) -> p wk s", s=8)
                    .unsqueeze(2).broadcast_to([128, 40, 8, 8]))
            ab1 = abv[:, :, 1].unsqueeze(2).broadcast_to([128, 8, 3, MUL])
            ab2 = abv[:, :, 2].unsqueeze(2).broadcast_to([128, 8, 5, MUL])
            nc.vector.tensor_tensor(l1v, yu_v, ab1, OP.mult)
            nc.vector.tensor_tensor(l2v, yy_v, ab2, OP.mult)

        def segment_phase(jj):
            nonlocal pair_i, oh_cur
            msgt = msg_tiles[jj]
            ab0 = (ab_tiles[jj][:].rearrange("p (w l c) -> p w l c",
                                             w=8, l=3)[:, :, 0])
            while pair_i < len(pairs) and pairs[pair_i][0] // 8 == jj:
                w, b, is_start, is_stop = pairs[pair_i]
                wj = w % 8
                gi, gs = divmod(pair_i, 8)
                if gs == 0:
                    oht = ohp.tile([128, 8 * 128], BF16, tag="oh", bufs=3)
                    n_in = min(8 * 128, (n_pairs - gi * 8) * 128)
                    nc.sync.dma_start(
                        oht[:, :n_in],
                        ohs_d[:, gi * 8 * 128:gi * 8 * 128 + n_in])
                    oh_cur = oht
                if is_start:
                    agg_a[b] = aggp.tile([128, 64], F32, tag="agg_a",
                                         name=f"agga{b}")
                    agg_b[b] = aggp.tile([128, 512], F32, tag="agg_b",
                                         name=f"aggb{b}")
                ata, atb = agg_a[b], agg_b[b]
                lhs = oh_cur[:, gs * 128:(gs + 1) * 128]
                nc.tensor.matmul(ata[:], lhs, ab0[:, wj],
                                 start=is_start, stop=is_stop)
                nc.tensor.matmul(atb[:], lhs,
                                 msgt[:, wj * 512:(wj + 1) * 512],
                                 start=is_start, stop=is_stop)
                if is_stop:
                    ot = outp.tile([128, MSG_W], F32, tag="ot")
                    nc.scalar.activation(ot[:, 0:MUL], ata[:], AF.Copy)
                    nc.scalar.activation(ot[:, MUL:MSG_W], atb[:], AF.Copy)
                    nc.sync.dma_start(
                        out_d[b * 128:(b + 1) * 128, :], ot[:])
                pair_i += 1

        oh_cur = None
        for j in range(CH + LOOKAHEAD):
            if j < CH:
                chunk_body(j)
            if j >= LOOKAHEAD:
                segment_phase(j - LOOKAHEAD)
        # empty blocks (defensive): write zeros
        empty = [b for b in range(BLOCKS) if meta["B_HI"][b] < meta["B_LO"][b]]
        if empty:
            zt = const.tile([128, MSG_W], F32)
            nc.vector.memset(zt[:], 0.0)
            for b in empty:
                nc.sync.dma_start(out_d[b * 128:(b + 1) * 128, :], zt[:])
    nc.compile()
    return nc


def _unpermute(out):
    """Device msg columns are K-MAJOR per l-block; restore reference order."""
    N = out.shape[0]
    l0 = out[:, 0:MUL]
    l1 = out[:, MUL:4 * MUL].reshape(N, 3, MUL).transpose(0, 2, 1)
    l2 = out[:, 4 * MUL:9 * MUL].reshape(N, 5, MUL).transpose(0, 2, 1)
    return np.concatenate(
        [l0, l1.reshape(N, 3 * MUL), l2.reshape(N, 5 * MUL)], axis=1)


def kernel(**inputs) -> np.ndarray:
    in_maps, meta = _prep(**inputs)
    nc = _build(meta)
    from concourse.bass_utils import run_bass_kernel_spmd
    res = run_bass_kernel_spmd(nc, in_maps, list(range(N_CORES)))
    outs = [np.asarray(res.results[c]["out"], np.float32)
            for c in range(N_CORES)]
    return _unpermute(np.concatenate(outs, axis=0))


if __name__ == "__main__":
    import reference
    ins = {k: np.asarray(v) for k, v in reference.setup_inputs().items()}
    out = kernel(**ins)
    exp = np.asarray(reference.reference(**reference.setup_inputs()))
    err = np.abs(out - exp).max() / np.abs(exp).max()
    print("rel err:", err)


# revision 29
# speedup vs baseline: 1.0007x; 1.0007x over previous
"""Trainium2 Bass kernel for MACE-style message-passing convolution.

Reference computation (per edge e with sender s, receiver r):
    msg0 = node_feats[s]                          # [64] scalars
    u    = vectors[e] / |vectors[e]|
    Y1   = sqrt(3) u ;  Y2 = 5 quadratic harmonics of u
    mix  = MLP(radial[e])                         # [192] = m0|m1|m2
    msg  = [msg0*m0, (msg0 (x) Y1)*m1, (msg0 (x) Y2)*m2]   # [576]
    out[r] += msg / sqrt(16)

Strategy (8 NeuronCores, SPMD):
  * Host: sort edges by receiver, shard NODES across cores (core c owns
    nodes [2048c, 2048(c+1))) so each core gets a contiguous slice of
    sorted edges -> no collective needed.
  * Host bakes data layouts only (gather of node_feats by sender, edge
    window layout, one-hot scatter matrices, weight folding). All FLOPs
    of the reference run on-device.
  * Device per core, per 1024-edge chunk (8 windows of 128 edges):
    - feature-major MLP (bf16 matmuls, silu on ACT)
    - edge-major mix via stationary-activations matmul (bf16)
    - tensor product on DVE in 2x packed mode: msg columns are K-MAJOR
      per window (k*64+c; host un-permutes at the end) so the Y factor
      is read from a small 8-slot replicated tile (built once per 4
      chunks on ACT; GPSIMD is avoided because it shares an SBUF port
      with the DVE and demotes concurrent 2-port DVE ops to 1x) and
      every AP keeps long unit-stride innermost runs -> DVE 2x
      everywhere.
    - segment-sum via one-hot bf16 matmuls with CONTIGUOUS moving
      operands (strided rhs runs 4x slow on PE), accumulating in PSUM
      per 128-node block.
    - mix-PSUM evacuation split between ACT (g0) and a fused DVE
      multiply straight from PSUM (g1) to balance engines.
"""

import os
import sys
from contextlib import ExitStack

import numpy as np

sys.path.insert(0, "/opt/trn_rl_repo")

import ml_dtypes  # noqa: E402

import concourse.bass as bass  # noqa: E402
import concourse.bacc as bacc  # noqa: E402
import concourse.tile as tile  # noqa: E402
from concourse import mybir  # noqa: E402

N_CORES = 8
N_NODES = 16384
N_EDGES = 262144
MUL = 64
N_BASIS = 8
HIDDEN = 64
NUM_IRREPS = 3 * MUL  # 192
MSG_W = 9 * MUL  # 576
NODES_PER_CORE = N_NODES // N_CORES  # 2048
BLOCKS = NODES_PER_CORE // 128  # 16 node-blocks of 128
WIN = 128  # edges per window (matmul K)
CHUNK_E = 1024  # edges per MLP chunk (2 stacked groups of 512)

F32 = mybir.dt.float32
F16 = mybir.dt.float16
BF16 = mybir.dt.bfloat16
F32R = mybir.dt.float32r
AF = mybir.ActivationFunctionType
OP = mybir.AluOpType

BF = ml_dtypes.bfloat16


def _silu_norm():
    x = np.linspace(-12.0, 12.0, 24001)
    p = np.exp(-0.5 * x * x) / np.sqrt(2.0 * np.pi)
    s = x / (1.0 + np.exp(-x))
    trapz = getattr(np, "trapz", None) or np.trapezoid
    return float(1.0 / np.sqrt(trapz(s * s * p, x)))


def _prep(vectors, node_feats, radial_embedding, W0, W1, W2, W3,
          senders, receivers):
    """Host-side data marshaling: sort/shard/pad/bake layouts."""
    E = vectors.shape[0]
    snd = np.asarray(senders).astype(np.int64)
    rcv = np.asarray(receivers).astype(np.int64)
    vectors = np.asarray(vectors, dtype=np.float32)
    node_feats = np.asarray(node_feats, dtype=np.float32)
    radial = np.asarray(radial_embedding, dtype=np.float32)

    perm = np.argsort(rcv, kind="stable")
    rcv_s = rcv[perm]
    snd_s = snd[perm]
    v_s = vectors[perm]
    rad_s = radial[perm]

    bounds = np.searchsorted(rcv_s, np.arange(N_CORES + 1) * NODES_PER_CORE)
    e_counts = np.diff(bounds)
    E_pad = int(np.ceil(e_counts.max() / CHUNK_E) * CHUNK_E)
    W = E_pad // WIN  # windows per core
    CH = E_pad // CHUNK_E

    sn = _silu_norm()
    W0e = (np.asarray(W0, np.float32) / np.sqrt(N_BASIS))
    W1e = (np.asarray(W1, np.float32) * sn / np.sqrt(HIDDEN))
    W2e = (np.asarray(W2, np.float32) * sn / np.sqrt(HIDDEN))
    W3e = (np.asarray(W3, np.float32) * sn / np.sqrt(HIDDEN) / 4.0).copy()
    W3e[:, MUL:2 * MUL] *= np.sqrt(3.0)  # fold Y1 = sqrt(3) u

    def blockdiag(w):
        k, m = w.shape
        out = np.zeros((2 * k, 2 * m), np.float32)
        out[:k, :m] = w
        out[k:, m:] = w
        return out

    w01 = blockdiag(W0e).astype(BF)
    w1b = blockdiag(W1e).astype(BF)
    w2b = blockdiag(W2e).astype(BF)
    w3e = W3e.astype(BF)

    # Per-core block->window ranges, unified across cores (SPMD: one program)
    core = {}
    blo_all = np.full((N_CORES, BLOCKS), 10**9, np.int64)
    bhi_all = np.full((N_CORES, BLOCKS), -1, np.int64)
    for c in range(N_CORES):
        lo, hi = bounds[c], bounds[c + 1]
        ec = hi - lo
        rl = rcv_s[lo:hi] - c * NODES_PER_CORE  # local node ids [0, 2048)
        rl_pad = np.full(E_pad, -1, np.int64)
        rl_pad[:ec] = rl
        # block edge ranges within this core's (padded) edge list
        bb = np.searchsorted(rl, np.arange(BLOCKS + 1) * 128)
        for b in range(BLOCKS):
            if bb[b + 1] > bb[b]:
                blo_all[c, b] = bb[b] // WIN
                bhi_all[c, b] = (bb[b + 1] - 1) // WIN
        core[c] = dict(lo=lo, hi=hi, ec=ec, rl_pad=rl_pad)
    B_LO = blo_all.min(axis=0)
    B_HI = bhi_all.max(axis=0)
    for b in range(BLOCKS):
        if B_HI[b] < B_LO[b]:
            B_LO[b], B_HI[b] = 0, -1  # empty everywhere -> memset path
    # pair list in window-major emission order
    pairs = []  # (w, b, start, stop)
    for w in range(W):
        for b in range(BLOCKS):
            if B_LO[b] <= w <= B_HI[b]:
                pairs.append((w, b, w == B_LO[b], w == B_HI[b]))
    n_pairs = len(pairs)

    in_maps = []
    for c in range(N_CORES):
        cc = core[c]
        lo, ec = cc["lo"], cc["ec"]
        # padded per-core edge arrays
        v_pad = np.zeros((E_pad, 3), np.float32)
        v_pad[:, 0] = 1.0
        v_pad[:ec] = v_s[lo:lo + ec]
        rad_pad = np.zeros((E_pad, N_BASIS), np.float32)
        rad_pad[:ec] = rad_s[lo:lo + ec]
        snd_pad = np.zeros(E_pad, np.int64)
        snd_pad[:ec] = snd_s[lo:lo + ec]

        # msg0 in per-chunk (w8, c64) layout: [128, CH*512]
        msg0 = node_feats[snd_pad]  # [E_pad, 64] host gather (layout only)
        msg0 = (msg0.reshape(CH, 8, 128, MUL).transpose(2, 0, 1, 3)
                .reshape(128, CH * 8 * MUL).astype(BF))

        # vectors in (k3, w) layout: [128, 3*W]
        vint = (v_pad.reshape(W, WIN, 3).transpose(1, 2, 0)
                .reshape(128, 3 * W))

        r4 = rad_pad.reshape(CH, 2, 512, N_BASIS)
        rad16 = np.ascontiguousarray(
            r4.transpose(1, 3, 0, 2).reshape(16, CH * 512)).astype(BF)

        # one-hot scatter matrices per (w, b) pair, bf16 (exact 0/1)
        rlp = cc["rl_pad"]
        ohs = np.zeros((n_pairs, WIN, 128), BF)
        ar = np.arange(128)
        for i, (w, b, _, _) in enumerate(pairs):
            rloc = rlp[w * WIN:(w + 1) * WIN] - 128 * b
            ohs[i] = (rloc[:, None] == ar[None, :]).astype(BF)
        ohs = ohs.transpose(1, 0, 2).reshape(WIN, n_pairs * 128)

        in_maps.append({
            "msg0": np.ascontiguousarray(msg0),
            "vint": np.ascontiguousarray(vint),
            "rad16": np.ascontiguousarray(rad16),
            "ohs": np.ascontiguousarray(ohs),
            "w01": w01, "w1b": w1b, "w2b": w2b, "w3e": w3e,
        })

    meta = dict(W=W, CH=CH, pairs=pairs, n_pairs=n_pairs,
                B_LO=B_LO, B_HI=B_HI)
    return in_maps, meta


def _build(meta, sim_safe=False):
    """Build the SPMD Bass/Tile program (identical across cores).

    sim_safe: CoreSim doesn't implement the Silu ACT function; emit
    Sigmoid + elementwise multiply instead (identical math) for sim runs.
    """
    W = meta["W"]
    CH = meta["CH"]
    pairs = meta["pairs"]
    n_pairs = meta["n_pairs"]

    nc = bacc.Bacc("TRN2", target_bir_lowering=False, debug=False)
    msg0_d = nc.declare_dram_parameter("msg0", [128, CH * 512], BF16,
                                       isOutput=False)
    vint_d = nc.declare_dram_parameter("vint", [128, 3 * W], F32,
                                       isOutput=False)
    rad_d = nc.declare_dram_parameter("rad16", [16, CH * 512], BF16,
                                      isOutput=False)
    ohs_d = nc.declare_dram_parameter("ohs", [128, n_pairs * 128], BF16,
                                      isOutput=False)
    w01_d = nc.declare_dram_parameter("w01", [16, 128], BF16, isOutput=False)
    w1b_d = nc.declare_dram_parameter("w1b", [128, 128], BF16, isOutput=False)
    w2b_d = nc.declare_dram_parameter("w2b", [128, 128], BF16, isOutput=False)
    w3e_d = nc.declare_dram_parameter("w3e", [64, NUM_IRREPS], BF16,
                                      isOutput=False)
    out_d = nc.declare_dram_parameter("out", [NODES_PER_CORE, MSG_W], F32,
                                      isOutput=True)

    C15 = float(np.sqrt(15.0))
    C5H = float(np.sqrt(5.0) / 2.0)

    def silu(out_ap, in_ap):
        if sim_safe:
            nc.scalar.activation(out_ap, in_ap, AF.Sigmoid)
            nc.vector.tensor_tensor(out_ap, out_ap, in_ap, OP.mult)
        else:
            nc.scalar.activation(out_ap, in_ap, AF.Silu)

    with tile.TileContext(nc) as tc, ExitStack() as ctx:
        const = ctx.enter_context(tc.tile_pool(name="const", bufs=1))
        sphp = ctx.enter_context(tc.tile_pool(name="sph", bufs=1))
        radp = ctx.enter_context(tc.tile_pool(name="rad", bufs=2))
        hp = ctx.enter_context(tc.tile_pool(name="hp", bufs=2, space="PSUM"))
        hact = ctx.enter_context(tc.tile_pool(name="hact", bufs=2))
        h3p = ctx.enter_context(tc.tile_pool(name="h3", bufs=2))
        mixp = ctx.enter_context(tc.tile_pool(name="mixp", bufs=1,
                                              space="PSUM"))
        mxsp = ctx.enter_context(tc.tile_pool(name="mxs", bufs=2))
        abp = ctx.enter_context(tc.tile_pool(name="ab", bufs=4))
        m0p = ctx.enter_context(tc.tile_pool(name="m0", bufs=3))
        yrp = ctx.enter_context(tc.tile_pool(name="yr", bufs=2))
        msgp = ctx.enter_context(tc.tile_pool(name="msg", bufs=4))
        ohp = ctx.enter_context(tc.tile_pool(name="oh", bufs=2))
        aggp = ctx.enter_context(tc.tile_pool(name="agg", bufs=2,
                                              space="PSUM"))
        outp = ctx.enter_context(tc.tile_pool(name="outs", bufs=2))

        # --- head DMAs: vint first (longest dependency chain: sph) ---
        vt = sphp.tile([128, 3 * W], F32)
        nc.sync.dma_start(vt[:], vint_d[:])
        w01t = const.tile([16, 128], BF16)
        nc.sync.dma_start(w01t[:], w01_d[:])
        w1bt = const.tile([128, 128], BF16)
        nc.sync.dma_start(w1bt[:], w1b_d[:])
        w2bt = const.tile([128, 128], BF16)
        nc.sync.dma_start(w2bt[:], w2b_d[:])
        # two copies of W3 (partitions 0:64 and 64:128) so the mix matmul's
        # lhsT (h3 slice) and rhs share a base partition
        w3et = const.tile([128, NUM_IRREPS], BF16)
        nc.sync.dma_start(w3et[0:64, :], w3e_d[:])
        nc.sync.dma_start(w3et[64:128, :], w3e_d[:])

        # --- PE warm-up: the HAM clock gate keeps the PE at 1.2 GHz until
        # it sees ~3.4us of sustained activity. The PE is otherwise idle
        # during the head DMAs + sph phase, so the real matmul stream would
        # start cold. Burn ~4us of dummy matmuls on the already-loaded
        # weights (recycling the MLP PSUM pool: no extra banks).
        for wi in range(80):
            wps = mixp.tile([128, 4 * 256], F32, tag="mixt",
                            name=f"warm{wi}")
            nc.tensor.matmul(wps[:, 0:128], w1bt[:], w2bt[:])

        # --- spherical harmonics, fp32, once over all windows ---
        # (k, w) layout: component-major, window innermost
        vsq = sphp.tile([128, 3 * W], F32)
        nc.vector.tensor_tensor(vsq[:], vt[:], vt[:], OP.mult)
        s2 = sphp.tile([128, W], F32)
        nc.vector.tensor_tensor(s2[:], vsq[:, 0:W], vsq[:, W:2 * W], OP.add)
        nc.vector.tensor_tensor(s2[:], s2[:], vsq[:, 2 * W:3 * W], OP.add)
        rs = sphp.tile([128, W], F32)
        nc.vector.reciprocal(rs[:], s2[:])
        rinv = sphp.tile([128, W], F32)  # 1/|v|
        nc.scalar.activation(rinv[:], rs[:], AF.Sqrt)
        u3 = sphp.tile([128, 3 * W], F32)
        nc.vector.tensor_tensor(
            u3[:].rearrange("p (k w) -> p k w", k=3),
            vt[:].rearrange("p (k w) -> p k w", k=3),
            rinv[:].unsqueeze(1).broadcast_to([128, 3, W]),
            OP.mult)
        ux = u3[:, 0:W]
        uy = u3[:, W:2 * W]
        uz = u3[:, 2 * W:3 * W]
        y5 = sphp.tile([128, 5 * W], F32)
        nc.vector.scalar_tensor_tensor(y5[:, 0:W], ux, C15, uy,
                                       OP.mult, OP.mult)
        nc.vector.scalar_tensor_tensor(y5[:, W:2 * W], uy, C15, uz,
                                       OP.mult, OP.mult)
        nc.vector.scalar_tensor_tensor(y5[:, 2 * W:3 * W], uz, 3.0 * C5H, uz,
                                       OP.mult, OP.mult)
        nc.vector.tensor_scalar_add(y5[:, 2 * W:3 * W], y5[:, 2 * W:3 * W],
                                    -C5H)
        nc.vector.scalar_tensor_tensor(y5[:, 3 * W:4 * W], ux, C15, uz,
                                       OP.mult, OP.mult)
        tpq = sphp.tile([128, 2 * W], F32)
        nc.vector.tensor_tensor(tpq[:, :W], ux, uy, OP.add)
        nc.vector.tensor_tensor(tpq[:, W:], ux, uy, OP.subtract)
        nc.vector.scalar_tensor_tensor(y5[:, 4 * W:5 * W], tpq[:, :W],
                                       C15 / 2.0, tpq[:, W:],
                                       OP.mult, OP.mult)

        # bf16 Y factors, all 8 components in one (k8, W) tile
        yall = sphp.tile([128, 8 * W], BF16)
        nc.vector.tensor_copy(yall[:, 0:3 * W], u3[:])
        nc.vector.tensor_copy(yall[:, 3 * W:8 * W], y5[:])
        yv = yall[:].rearrange("p (k w) -> p k w", k=8)

        # --- software-pipelined chunk loop ---
        LOOKAHEAD = 3
        pair_i = 0
        agg_a = {}
        agg_b = {}
        ab_tiles = {}
        msg_tiles = {}
        yrep = [None, None]

        def chunk_body(j):
            # MLP chunk j: 1024 edges as 2 stacked groups (even/odd windows
            # so consecutive mix matmuls alternate PE row-halves)
            radt = radp.tile([16, 512], BF16, tag="radt")
            nc.sync.dma_start(radt[:], rad_d[:, j * 512:(j + 1) * 512])
            h1ps = hp.tile([128, 512], F32, tag="hps")
            nc.tensor.matmul(h1ps[:], w01t[:], radt[:])
            h1 = hact.tile([128, 512], BF16, tag="h12")
            silu(h1[:], h1ps[:])
            h2ps = hp.tile([128, 512], F32, tag="hps")
            nc.tensor.matmul(h2ps[:], w1bt[:], h1[:])
            h2 = hact.tile([128, 512], BF16, tag="h12")
            silu(h2[:], h2ps[:])
            h3ps = hp.tile([128, 512], F32, tag="hps")
            nc.tensor.matmul(h3ps[:], w2bt[:], h2[:])
            h3 = h3p.tile([128, 512], BF16)
            silu(h3[:], h3ps[:])
            hoff = 0

            # msg0 chunk in (w8, c64) layout
            m0t = m0p.tile([128, 512], BF16)
            nc.sync.dma_start(
                m0t[:], msg0_d[:, j * 512:(j + 1) * 512])
            m0v = m0t[:].rearrange("p (w c) -> p w c", w=8)

            # 8-slot replicated Y factor tiles, built once per 4 chunks on
            # ACT (any-AP 1x; the big FD amortizes the per-op constant).
            # GPSIMD is NOT used: it shares an SBUF port with the DVE and
            # demotes concurrent 2-port (2x) DVE ops to 1x.
            if j % 4 == 0:
                wlo, whi = j * 8, min((j + 4) * 8, W)
                nw = whi - wlo
                yu8 = yrp.tile([128, 32 * 3 * 8], BF16, tag="yu8",
                               name=f"yu8_{j}")
                nc.scalar.activation(
                    yu8[:, :nw * 24].rearrange(
                        "p (w k s) -> p w k s", k=3, s=8),
                    yv[:, 0:3, wlo:whi].transpose([0, 2, 1])
                    .unsqueeze(3).broadcast_to([128, nw, 3, 8]),
                    AF.Copy)
                yy8 = yrp.tile([128, 32 * 5 * 8], BF16, tag="yy8",
                               name=f"yy8_{j}")
                nc.scalar.activation(
                    yy8[:, :nw * 40].rearrange(
                        "p (w k s) -> p w k s", k=5, s=8),
                    yv[:, 3:8, wlo:whi].transpose([0, 2, 1])
                    .unsqueeze(3).broadcast_to([128, nw, 5, 8]),
                    AF.Copy)
                yrep[0] = yu8
                yrep[1] = yy8

            # ab tile: (w8, l3, c64) layout
            ab = abp.tile([128, 8 * 3 * MUL], BF16, tag="ab",
                          name=f"ab_{j}", bufs=LOOKAHEAD + 2)
            ab_tiles[j] = ab
            abv = ab[:].rearrange("p (w l c) -> p w l c", w=8, l=3)

            for g in range(2):  # half-chunks of 4 windows
                # mix: edge-major via stationary-h3 trick; 256-col PSUM
                # slots so each matmul output stays inside one bank
                mixt = mixp.tile([128, 4 * 256], F32, tag="mixt")
                for t4 in range(4):
                    t = g * 4 + t4
                    half, coff = ((0, t * 128) if t < 4
                                  else (64, (t - 4) * 128))
                    nc.tensor.matmul(
                        mixt[:, t4 * 256:t4 * 256 + NUM_IRREPS],
                        h3[half:half + 64, coff:coff + 128],
                        w3et[half:half + 64, :])
                # PSUM mix view (w4, l, c)
                mixv = (mixt[:].rearrange("p (w x) -> p w x", x=256)
                        [:, :, 0:NUM_IRREPS]
                        .rearrange("p w (l c) -> p w l c", l=3))
                abw = abv[:, g * 4:(g + 1) * 4]
                m0w = (m0v[:, g * 4:(g + 1) * 4]
                       .unsqueeze(2).broadcast_to([128, 4, 3, MUL]))
                if g == 0:
                    # ACT evacuates PSUM -> bf16 SBUF (frees the single mix
                    # PSUM buffer fast); DVE multiplies at 2x
                    mixs = mxsp.tile([128, 4 * 3 * MUL], BF16, tag="mixs")
                    msv = mixs[:].rearrange("p (w l c) -> p w l c",
                                            w=4, l=3)
                    nc.scalar.activation(msv, mixv, AF.Copy)
                    nc.vector.tensor_tensor(abw, msv, m0w, OP.mult)
                else:
                    # fused: DVE reads mix from PSUM (1x) and multiplies
                    nc.vector.tensor_tensor(abw, mixv, m0w, OP.mult)

            # msg tile [128, 8*512]: (w8, col512) with K-MAJOR columns
            # col = k*64+c for l1 (0:192), 192 + k*64+c for l2 (192:512)
            msgt = msgp.tile([128, 8 * 512], BF16, bufs=LOOKAHEAD + 2)
            msg_tiles[j] = msgt
            l1v = (msgt[:].rearrange("p (w col) -> p w col", w=8)
                   [:, :, 0:192].rearrange("p w (k c) -> p w k c", k=3))
            l2v = (msgt[:].rearrange("p (w col) -> p w col", w=8)
                   [:, :, 192:512].rearrange("p w (k c) -> p w k c", k=5))
            ou = (j % 4) * 8 * 24
            oy = (j % 4) * 8 * 40
            yu_v = (yrep[0][:, ou:ou + 192]
                    .rearrange("p (wk s) -> p wk s", s=8)
                    .unsqueeze(2).broadcast_to([128, 24, 8, 8]))
            yy_v = (yrep[1][:, oy:oy + 320]
                    .rearrange("p (wk s) -> p wk s", s=8)
                    .unsqueeze(2).broadcast_to([128, 40, 8, 8]))
            ab1 = abv[:, :, 1].unsqueeze(2).broadcast_to([128, 8, 3, MUL])
            ab2 = abv[:, :, 2].unsqueeze(2).broadcast_to([128, 8, 5, MUL])
            nc.vector.tensor_tensor(l1v, yu_v, ab1, OP.mult)
            nc.vector.tensor_tensor(l2v, yy_v, ab2, OP.mult)

        def segment_phase(jj):
            nonlocal pair_i, oh_cur
            msgt = msg_tiles[jj]
            ab0 = (ab_tiles[jj][:].rearrange("p (w l c) -> p w l c",
                                             w=8, l=3)[:, :, 0])
            while pair_i < len(pairs) and pairs[pair_i][0] // 8 == jj:
                w, b, is_start, is_stop = pairs[pair_i]
                wj = w % 8
                gi, gs = divmod(pair_i, 8)
                if gs == 0:
                    oht = ohp.tile([128, 8 * 128], BF16, tag="oh", bufs=4)
                    n_in = min(8 * 128, (n_pairs - gi * 8) * 128)
                    nc.sync.dma_start(
                        oht[:, :n_in],
                        ohs_d[:, gi * 8 * 128:gi * 8 * 128 + n_in])
                    oh_cur = oht
                if is_start:
                    agg_a[b] = aggp.tile([128, 64], F32, tag="agg_a",
                                         name=f"agga{b}")
                    agg_b[b] = aggp.tile([128, 512], F32, tag="agg_b",
                                         name=f"aggb{b}")
                ata, atb = agg_a[b], agg_b[b]
                lhs = oh_cur[:, gs * 128:(gs + 1) * 128]
                nc.tensor.matmul(ata[:], lhs, ab0[:, wj],
                                 start=is_start, stop=is_stop)
                nc.tensor.matmul(atb[:], lhs,
                                 msgt[:, wj * 512:(wj + 1) * 512],
                                 start=is_start, stop=is_stop)
                if is_stop:
                    ot = outp.tile([128, MSG_W], F32, tag="ot")
                    nc.scalar.activation(ot[:, 0:MUL], ata[:], AF.Copy)
                    nc.scalar.activation(ot[:, MUL:MSG_W], atb[:], AF.Copy)
                    nc.sync.dma_start(
                        out_d[b * 128:(b + 1) * 128, :], ot[:])
                pair_i += 1

        oh_cur = None
        for j in range(CH + LOOKAHEAD):
            if j < CH:
                chunk_body(j)
            if j >= LOOKAHEAD:
                segment_phase(j - LOOKAHEAD)
        # empty blocks (defensive): write zeros
        empty = [b for b in range(BLOCKS) if meta["B_HI"][b] < meta["B_LO"][b]]
        if empty:
            zt = const.tile([128, MSG_W], F32)
            nc.vector.memset(zt[:], 0.0)
            for b in empty:
                nc.sync.dma_start(out_d[b * 128:(b + 1) * 128, :], zt[:])
    nc.compile()
    return nc


def _unpermute(out):
    """Device msg columns are K-MAJOR per l-block; restore reference order."""
    N = out.shape[0]
    l0 = out[:, 0:MUL]
    l1 = out[:, MUL:4 * MUL].reshape(N, 3, MUL).transpose(0, 2, 1)
    l2 = out[:, 4 * MUL:9 * MUL].reshape(N, 5, MUL).transpose(0, 2, 1)
    return np.concatenate(
        [l0, l1.reshape(N, 3 * MUL), l2.reshape(N, 5 * MUL)], axis=1)


def kernel(**inputs) -> np.ndarray:
    in_maps, meta = _prep(**inputs)
    nc = _build(meta)
    from concourse.bass_utils import run_bass_kernel_spmd
    res = run_bass_kernel_spmd(nc, in_maps, list(range(N_CORES)))
    outs = [np.asarray(res.results[c]["out"], np.float32)
            for c in range(N_CORES)]
    return _unpermute(np.concatenate(outs, axis=0))


if __name__ == "__main__":
    import reference
    ins = {k: np.asarray(v) for k, v in reference.setup_inputs().items()}
    out = kernel(**ins)
    exp = np.asarray(reference.reference(**reference.setup_inputs()))
    err = np.abs(out - exp).max() / np.abs(exp).max()
    print("rel err:", err)
